# revision 12
# baseline (speedup 1.0000x reference)
"""Trainium2 Bass kernel for nn_Metric_35588099015209.

Reference computation (BS=64, IN_DIM=9801, OUT_DIM=25):
    d      = einsum('oi,bi->bo', measure,     input)   # (64, 25)
    d_full = einsum('fi,bi->bf', fullmeasure, input)   # (64, 9801)

Structure exploited (guaranteed by the Metric module's construction and
verified at runtime before use): `measure` is 25 distinct rows of the
identity, and `fullmeasure` is the identity with the complementary rows
zeroed — a diagonal 0/1 matrix whose 25 ones sit exactly at the columns
`measure` selects. Therefore:
    d[b, o]      = input[b, idx[o]]               (a 25-column gather)
    d_full[b, f] = input[b, f] * mask[f]          (an elementwise mask)

Sharding: data-parallel over the batch — each of the 8 NeuronCores gets 8
rows of `input` and produces its 8 rows of both outputs; the mask / index
constants are replicated. No communication.

Per-core kernel (raw Bass, explicit single-wait semaphores — this
toolchain's walrus rejects instructions carrying more than one sem-wait;
DMA-issue cost and DMA-completion latency dominate at this size, so the
design minimizes dma_start count; 9801 = 11*891 gives an exact 2D SBUF
layout with partition = chunk*8 + batch_row and a single 3D-access-pattern
store for the whole d_full image):
  SP  : load xz (88,891); after mul half0: store columns [0,445) of every
        chunk; after gather: store d; final completion wait
  Act : load idx table; load mz mask image; after mul half1: store columns
        [445,891)
  DVE : elementwise t_xz *= t_mz in two column halves (store of half 0
        overlaps multiply of half 1)
  Pool: indirect-DMA gather of the 25 live rows of xt (9801,8) -> d tile
"""

import numpy as np

IN_DIM = 9801
OUT_DIM = 25
BS = 64
N_CORES = 8
BPC = BS // N_CORES  # batch rows per core
NCHUNK = 11
CHUNK = 891  # 11 * 891 = 9801 exactly
NPART = NCHUNK * BPC  # 88 partitions


def _build_program():
    import concourse.bass as bass
    import concourse.mybir as mybir
    from contextlib import ExitStack

    f32 = mybir.dt.float32
    i32 = mybir.dt.int32
    nc = bass.Bass()
    # xz and mz packed side-by-side: one load, one completion semaphore
    xzmz = nc.dram_tensor("xzmz", [NPART, 2 * CHUNK], f32, kind="ExternalInput")
    xt = nc.dram_tensor("xt", [IN_DIM, BPC], f32, kind="ExternalInput")
    idxt = nc.dram_tensor("idx", [OUT_DIM, 1], i32, kind="ExternalInput")
    d = nc.dram_tensor("d", [OUT_DIM, BPC], f32, kind="ExternalOutput")
    dfull = nc.dram_tensor("dfull", [BPC, IN_DIM], f32, kind="ExternalOutput")

    with ExitStack() as stack:
        t_idx = stack.enter_context(nc.sbuf_tensor([OUT_DIM, 1], i32))
        t_g = stack.enter_context(nc.sbuf_tensor([OUT_DIM, BPC], f32))
        t = stack.enter_context(nc.sbuf_tensor([NPART, 2 * CHUNK], f32))
        s_idx = stack.enter_context(nc.semaphore("s_idx"))
        s_g = stack.enter_context(nc.semaphore("s_g"))
        s_fin = stack.enter_context(nc.semaphore("s_fin"))
        s_ld0 = stack.enter_context(nc.semaphore("s_ld0"))
        s_ld1 = stack.enter_context(nc.semaphore("s_ld1"))
        s_mul = stack.enter_context(nc.semaphore("s_mul"))
        block = stack.enter_context(nc.Block(no_gpsimd_drain=True))

        # Column halves of the d_full image: J0=445, J1=446 (891 = 445+446).
        # The packed input row is [xz_j0 | mz_j0 | xz_j1 | mz_j1], so each
        # half is one contiguous load and its multiply starts as soon as
        # that half's transfer completes — not when the whole image lands.
        J0 = CHUNK // 2
        J1 = CHUNK - J0
        H0 = 2 * J0  # 890: end of half-0 pack
        # d_full viewed as (chunk, batch, j): matches t's element order
        # (the SBUF side must stay a plain 2D AP — split-partition SBUF APs
        # mis-lower in this toolchain).
        dram3d = dfull[:, :].rearrange("b (c j) -> c b j", j=CHUNK)

        @block.sync
        def _(sync):
            # idx first: its 100 B transfer must not queue behind the loads
            sync.dma_start(out=t_idx[:], in_=idxt[:]).then_inc(s_idx, 16)
            sync.dma_start(out=t[:, :H0], in_=xzmz[:, :H0]).then_inc(s_ld0, 16)
            sync.wait_ge(s_mul, 1)
            sync.dma_start(out=dram3d[:, :, :J0], in_=t[:, :J0]).then_inc(
                s_fin, 16
            )
            sync.wait_ge(s_g, 16)
            sync.dma_start(out=d[:], in_=t_g[:]).then_inc(s_fin, 16)
            sync.wait_ge(s_fin, 48)

        @block.scalar
        def _(scalar):
            scalar.dma_start(out=t[:, H0:], in_=xzmz[:, H0:]).then_inc(s_ld1, 16)
            scalar.wait_ge(s_mul, 2)
            scalar.dma_start(
                out=dram3d[:, :, J0:], in_=t[:, H0 : H0 + J1]
            ).then_inc(s_fin, 16)

        @block.vector
        def _(vector):
            vector.wait_ge(s_ld0, 16)
            vector.tensor_mul(
                out=t[:, :J0], in0=t[:, :J0], in1=t[:, J0:H0]
            ).then_inc(s_mul, 1)
            vector.wait_ge(s_ld1, 16)
            vector.tensor_mul(
                out=t[:, H0 : H0 + J1],
                in0=t[:, H0 : H0 + J1],
                in1=t[:, H0 + J1 :],
            ).then_inc(s_mul, 1)

        @block.gpsimd
        def _(gpsimd):
            gpsimd.wait_ge(s_idx, 16)
            gpsimd.indirect_dma_start(
                out=t_g[:],
                out_offset=None,
                in_=xt[:],
                in_offset=bass.IndirectOffsetOnAxis(ap=t_idx[:, :1], axis=0),
            ).then_inc(s_g, 16)

    return nc


_PROGRAM_CACHE: dict = {}


def _derive_structure(measure, fullmeasure):
    """Extract the 25 selected columns; verify the structural assumptions
    the kernel relies on. Returns None if they don't hold."""
    measure = np.asarray(measure, dtype=np.float32)
    if measure.shape != (OUT_DIM, IN_DIM):
        return None
    idx = measure.argmax(axis=1)
    onehot = np.zeros_like(measure)
    onehot[np.arange(OUT_DIM), idx] = 1.0
    if not np.array_equal(measure, onehot):
        return None
    diag = np.asarray(np.diagonal(fullmeasure), dtype=np.float32)
    expect = np.zeros(IN_DIM, dtype=np.float32)
    expect[idx] = 1.0
    if not np.array_equal(diag, expect):
        return None
    return tuple(int(f) for f in idx)


def kernel(input, measure, fullmeasure):
    from concourse.bass_utils import run_bass_kernel_spmd

    x = np.ascontiguousarray(np.asarray(input, dtype=np.float32))
    assert x.shape == (BS, IN_DIM), x.shape

    idx = _derive_structure(measure, fullmeasure)
    if idx is None:
        # Structure violated (cannot happen with the Metric construction):
        # fall back to the dense definition so the result is still correct.
        m = np.asarray(measure, dtype=np.float32)
        fm = np.asarray(fullmeasure, dtype=np.float32)
        return (x @ m.T, x @ fm.T)

    if "nc" not in _PROGRAM_CACHE:
        _PROGRAM_CACHE["nc"] = _build_program()
    nc = _PROGRAM_CACHE["nc"]

    mask = np.zeros(IN_DIM, dtype=np.float32)
    mask[list(idx)] = 1.0
    # mask in the (chunk*BPC + b, j) SBUF layout; identical for every b
    mz = np.ascontiguousarray(
        np.broadcast_to(
            mask.reshape(NCHUNK, 1, CHUNK), (NCHUNK, BPC, CHUNK)
        ).reshape(NPART, CHUNK)
    )
    idx_arr = np.asarray(idx, dtype=np.int32).reshape(OUT_DIM, 1)

    J0 = CHUNK // 2
    in_maps = []
    for k in range(N_CORES):
        shard = x[k * BPC : (k + 1) * BPC, :]  # (8, 9801)
        xz = shard.reshape(BPC, NCHUNK, CHUNK).transpose(1, 0, 2).reshape(
            NPART, CHUNK
        )
        # pack per row: [xz_j0 | mz_j0 | xz_j1 | mz_j1]
        xzmz = np.ascontiguousarray(
            np.concatenate(
                [xz[:, :J0], mz[:, :J0], xz[:, J0:], mz[:, J0:]], axis=1
            )
        )
        xt = np.ascontiguousarray(shard.T)
        in_maps.append({"xzmz": xzmz, "xt": xt, "idx": idx_arr})

    res = run_bass_kernel_spmd(nc, in_maps, core_ids=list(range(N_CORES)))
    d = np.concatenate([res.results[k]["d"].T for k in range(N_CORES)], axis=0)
    d_full = np.concatenate(
        [res.results[k]["dfull"] for k in range(N_CORES)], axis=0
    )
    return (d, d_full)


# revision 14
# speedup vs baseline: 1.0386x; 1.0386x over previous
"""Trainium2 Bass kernel for nn_Metric_35588099015209.

Reference computation (BS=64, IN_DIM=9801, OUT_DIM=25):
    d      = einsum('oi,bi->bo', measure,     input)   # (64, 25)
    d_full = einsum('fi,bi->bf', fullmeasure, input)   # (64, 9801)

Structure exploited (guaranteed by the Metric module's construction and
verified at runtime before use): `measure` is 25 distinct rows of the
identity, and `fullmeasure` is the identity with the complementary rows
zeroed — a diagonal 0/1 matrix whose 25 ones sit exactly at the columns
`measure` selects. Therefore:
    d[b, o]      = input[b, idx[o]]               (a 25-column gather)
    d_full[b, f] = input[b, f] * mask[f]          (an elementwise mask)

Sharding: data-parallel over the batch — each of the 8 NeuronCores gets 8
rows of `input` and produces its 8 rows of both outputs; the mask / index
constants are replicated. No communication.

Per-core kernel (raw Bass, explicit single-wait semaphores — this
toolchain's walrus rejects instructions carrying more than one sem-wait;
DMA-issue cost and DMA-completion latency dominate at this size, so the
design minimizes dma_start count; 9801 = 11*891 gives an exact 2D SBUF
layout with partition = chunk*8 + batch_row and a single 3D-access-pattern
store for the whole d_full image):
  SP  : load xz (88,891); after mul half0: store columns [0,445) of every
        chunk; after gather: store d; final completion wait
  Act : load idx table; load mz mask image; after mul half1: store columns
        [445,891)
  DVE : elementwise t_xz *= t_mz in two column halves (store of half 0
        overlaps multiply of half 1)
  Pool: indirect-DMA gather of the 25 live rows of xt (9801,8) -> d tile
"""

import numpy as np

IN_DIM = 9801
OUT_DIM = 25
BS = 64
N_CORES = 8
BPC = BS // N_CORES  # batch rows per core
NCHUNK = 11
CHUNK = 891  # 11 * 891 = 9801 exactly
NPART = NCHUNK * BPC  # 88 partitions


def _build_program():
    import concourse.bass as bass
    import concourse.mybir as mybir
    from contextlib import ExitStack

    f32 = mybir.dt.float32
    i32 = mybir.dt.int32
    nc = bass.Bass()
    # xz and mz packed side-by-side: one load, one completion semaphore
    xzmz = nc.dram_tensor("xzmz", [NPART, 2 * CHUNK], f32, kind="ExternalInput")
    xt = nc.dram_tensor("xt", [IN_DIM, BPC], f32, kind="ExternalInput")
    idxt = nc.dram_tensor("idx", [OUT_DIM, 1], i32, kind="ExternalInput")
    d = nc.dram_tensor("d", [OUT_DIM, BPC], f32, kind="ExternalOutput")
    dfull = nc.dram_tensor("dfull", [BPC, IN_DIM], f32, kind="ExternalOutput")

    with ExitStack() as stack:
        t_idx = stack.enter_context(nc.sbuf_tensor([OUT_DIM, 1], i32))
        t_g = stack.enter_context(nc.sbuf_tensor([OUT_DIM, BPC], f32))
        t = stack.enter_context(nc.sbuf_tensor([NPART, 2 * CHUNK], f32))
        s_idx = stack.enter_context(nc.semaphore("s_idx"))
        s_g = stack.enter_context(nc.semaphore("s_g"))
        s_fin = stack.enter_context(nc.semaphore("s_fin"))
        s_ld0 = stack.enter_context(nc.semaphore("s_ld0"))
        s_ld1 = stack.enter_context(nc.semaphore("s_ld1"))
        s_mul = stack.enter_context(nc.semaphore("s_mul"))

        # Column halves of the d_full image: J0=445, J1=446 (891 = 445+446).
        # The packed input row is [xz_j0 | mz_j0 | xz_j1 | mz_j1], so each
        # half is one contiguous load and its multiply starts as soon as
        # that half's transfer completes — not when the whole image lands.
        J0 = CHUNK // 2
        J1 = CHUNK - J0
        H0 = 2 * J0  # 890: end of half-0 pack
        # d_full viewed as (chunk, batch, j): matches t's element order
        # (the SBUF side must stay a plain 2D AP — split-partition SBUF APs
        # mis-lower in this toolchain).
        dram3d = dfull[:, :].rearrange("b (c j) -> c b j", j=CHUNK)

        # No Block(): emit the per-engine streams straight into the root
        # basic block — the program then ends at SP's final wait, with no
        # block-entry branches and no block-exit drain + all-engine
        # barrier tail (~0.5 us saved; interleaving across engines in the
        # bb is irrelevant, only per-engine order matters).
        # idx first: its 100 B transfer must not queue behind the loads.
        nc.sync.dma_start(out=t_idx[:], in_=idxt[:]).then_inc(s_idx, 16)
        nc.sync.dma_start(out=t[:, :H0], in_=xzmz[:, :H0]).then_inc(s_ld0, 16)
        nc.scalar.dma_start(out=t[:, H0:], in_=xzmz[:, H0:]).then_inc(s_ld1, 16)

        nc.gpsimd.wait_ge(s_idx, 16)
        nc.gpsimd.indirect_dma_start(
            out=t_g[:],
            out_offset=None,
            in_=xt[:],
            in_offset=bass.IndirectOffsetOnAxis(ap=t_idx[:, :1], axis=0),
        ).then_inc(s_g, 16)

        nc.vector.wait_ge(s_ld0, 16)
        nc.vector.tensor_mul(
            out=t[:, :J0], in0=t[:, :J0], in1=t[:, J0:H0]
        ).then_inc(s_mul, 1)
        nc.vector.wait_ge(s_ld1, 16)
        nc.vector.tensor_mul(
            out=t[:, H0 : H0 + J1],
            in0=t[:, H0 : H0 + J1],
            in1=t[:, H0 + J1 :],
        ).then_inc(s_mul, 1)

        nc.sync.wait_ge(s_mul, 1)
        nc.sync.dma_start(out=dram3d[:, :, :J0], in_=t[:, :J0]).then_inc(
            s_fin, 16
        )
        nc.scalar.wait_ge(s_mul, 2)
        nc.scalar.dma_start(
            out=dram3d[:, :, J0:], in_=t[:, H0 : H0 + J1]
        ).then_inc(s_fin, 16)
        nc.sync.wait_ge(s_g, 16)
        nc.sync.dma_start(out=d[:], in_=t_g[:]).then_inc(s_fin, 16)
        nc.sync.wait_ge(s_fin, 48)

    return nc


_PROGRAM_CACHE: dict = {}


def _derive_structure(measure, fullmeasure):
    """Extract the 25 selected columns; verify the structural assumptions
    the kernel relies on. Returns None if they don't hold."""
    measure = np.asarray(measure, dtype=np.float32)
    if measure.shape != (OUT_DIM, IN_DIM):
        return None
    idx = measure.argmax(axis=1)
    onehot = np.zeros_like(measure)
    onehot[np.arange(OUT_DIM), idx] = 1.0
    if not np.array_equal(measure, onehot):
        return None
    diag = np.asarray(np.diagonal(fullmeasure), dtype=np.float32)
    expect = np.zeros(IN_DIM, dtype=np.float32)
    expect[idx] = 1.0
    if not np.array_equal(diag, expect):
        return None
    return tuple(int(f) for f in idx)


def kernel(input, measure, fullmeasure):
    from concourse.bass_utils import run_bass_kernel_spmd

    x = np.ascontiguousarray(np.asarray(input, dtype=np.float32))
    assert x.shape == (BS, IN_DIM), x.shape

    idx = _derive_structure(measure, fullmeasure)
    if idx is None:
        # Structure violated (cannot happen with the Metric construction):
        # fall back to the dense definition so the result is still correct.
        m = np.asarray(measure, dtype=np.float32)
        fm = np.asarray(fullmeasure, dtype=np.float32)
        return (x @ m.T, x @ fm.T)

    if "nc" not in _PROGRAM_CACHE:
        _PROGRAM_CACHE["nc"] = _build_program()
    nc = _PROGRAM_CACHE["nc"]

    mask = np.zeros(IN_DIM, dtype=np.float32)
    mask[list(idx)] = 1.0
    # mask in the (chunk*BPC + b, j) SBUF layout; identical for every b
    mz = np.ascontiguousarray(
        np.broadcast_to(
            mask.reshape(NCHUNK, 1, CHUNK), (NCHUNK, BPC, CHUNK)
        ).reshape(NPART, CHUNK)
    )
    idx_arr = np.asarray(idx, dtype=np.int32).reshape(OUT_DIM, 1)

    J0 = CHUNK // 2
    in_maps = []
    for k in range(N_CORES):
        shard = x[k * BPC : (k + 1) * BPC, :]  # (8, 9801)
        xz = shard.reshape(BPC, NCHUNK, CHUNK).transpose(1, 0, 2).reshape(
            NPART, CHUNK
        )
        # pack per row: [xz_j0 | mz_j0 | xz_j1 | mz_j1]
        xzmz = np.ascontiguousarray(
            np.concatenate(
                [xz[:, :J0], mz[:, :J0], xz[:, J0:], mz[:, J0:]], axis=1
            )
        )
        xt = np.ascontiguousarray(shard.T)
        in_maps.append({"xzmz": xzmz, "xt": xt, "idx": idx_arr})

    res = run_bass_kernel_spmd(nc, in_maps, core_ids=list(range(N_CORES)))
    d = np.concatenate([res.results[k]["d"].T for k in range(N_CORES)], axis=0)
    d_full = np.concatenate(
        [res.results[k]["dfull"] for k in range(N_CORES)], axis=0
    )
    return (d, d_full)


# revision 16
# speedup vs baseline: 1.0455x; 1.0067x over previous
"""Trainium2 Bass kernel for nn_Metric_35588099015209.

Reference computation (BS=64, IN_DIM=9801, OUT_DIM=25):
    d      = einsum('oi,bi->bo', measure,     input)   # (64, 25)
    d_full = einsum('fi,bi->bf', fullmeasure, input)   # (64, 9801)

Structure exploited (guaranteed by the Metric module's construction and
verified at runtime before use): `measure` is 25 distinct rows of the
identity, and `fullmeasure` is the identity with the complementary rows
zeroed — a diagonal 0/1 matrix whose 25 ones sit exactly at the columns
`measure` selects. Therefore:
    d[b, o]      = input[b, idx[o]]               (a 25-column gather)
    d_full[b, f] = input[b, f] * mask[f]          (an elementwise mask)

Sharding: data-parallel over the batch — each of the 8 NeuronCores gets 8
rows of `input` and produces its 8 rows of both outputs; the mask / index
constants are replicated. No communication.

Per-core kernel (raw Bass, explicit single-wait semaphores — this
toolchain's walrus rejects instructions carrying more than one sem-wait;
DMA-issue cost and DMA-completion latency dominate at this size, so the
design minimizes dma_start count; 9801 = 11*891 gives an exact 2D SBUF
layout with partition = chunk*8 + batch_row and a single 3D-access-pattern
store for the whole d_full image):
  SP  : load xz (88,891); after mul half0: store columns [0,445) of every
        chunk; after gather: store d; final completion wait
  Act : load idx table; load mz mask image; after mul half1: store columns
        [445,891)
  DVE : elementwise t_xz *= t_mz in two column halves (store of half 0
        overlaps multiply of half 1)
  Pool: indirect-DMA gather of the 25 live rows of xt (9801,8) -> d tile
"""

import numpy as np

IN_DIM = 9801
OUT_DIM = 25
BS = 64
N_CORES = 8
BPC = BS // N_CORES  # batch rows per core
NCHUNK = 11
CHUNK = 891  # 11 * 891 = 9801 exactly
NPART = NCHUNK * BPC  # 88 partitions


def _build_program():
    import concourse.bass as bass
    import concourse.mybir as mybir
    from contextlib import ExitStack

    f32 = mybir.dt.float32
    i32 = mybir.dt.int32
    nc = bass.Bass()
    # xz and mz packed side-by-side: one load, one completion semaphore
    xzmz = nc.dram_tensor("xzmz", [NPART, 2 * CHUNK], f32, kind="ExternalInput")
    xt = nc.dram_tensor("xt", [IN_DIM, BPC], f32, kind="ExternalInput")
    idxt = nc.dram_tensor("idx", [OUT_DIM, 1], i32, kind="ExternalInput")
    d = nc.dram_tensor("d", [OUT_DIM, BPC], f32, kind="ExternalOutput")
    dfull = nc.dram_tensor("dfull", [BPC, IN_DIM], f32, kind="ExternalOutput")

    with ExitStack() as stack:
        t_idx = stack.enter_context(nc.sbuf_tensor([OUT_DIM, 1], i32))
        t_g = stack.enter_context(nc.sbuf_tensor([OUT_DIM, BPC], f32))
        t = stack.enter_context(nc.sbuf_tensor([NPART, 2 * CHUNK], f32))
        s_idx = stack.enter_context(nc.semaphore("s_idx"))
        s_g = stack.enter_context(nc.semaphore("s_g"))
        s_fin = stack.enter_context(nc.semaphore("s_fin"))
        s_ld0 = stack.enter_context(nc.semaphore("s_ld0"))
        s_ld1 = stack.enter_context(nc.semaphore("s_ld1"))
        s_mul = stack.enter_context(nc.semaphore("s_mul"))

        # Column halves of the d_full image: J0=445, J1=446 (891 = 445+446).
        # The packed input row is [xz_j0 | mz_j0 | xz_j1 | mz_j1], so each
        # half is one contiguous load and its multiply starts as soon as
        # that half's transfer completes — not when the whole image lands.
        J0 = CHUNK // 2
        J1 = CHUNK - J0
        H0 = 2 * J0  # 890: end of half-0 pack
        # d_full viewed as (chunk, batch, j): matches t's element order
        # (the SBUF side must stay a plain 2D AP — split-partition SBUF APs
        # mis-lower in this toolchain).
        dram3d = dfull[:, :].rearrange("b (c j) -> c b j", j=CHUNK)

        # No Block(): emit the per-engine streams straight into the root
        # basic block — the program then ends at SP's final wait, with no
        # block-entry branches and no block-exit drain + all-engine
        # barrier tail (~0.5 us saved; interleaving across engines in the
        # bb is irrelevant, only per-engine order matters).
        # idx first: its 100 B transfer must not queue behind the loads.
        nc.sync.dma_start(out=t_idx[:], in_=idxt[:]).then_inc(s_idx, 16)
        nc.sync.dma_start(out=t[:, :H0], in_=xzmz[:, :H0]).then_inc(s_ld0, 16)
        nc.scalar.dma_start(out=t[:, H0:], in_=xzmz[:, H0:]).then_inc(s_ld1, 16)

        nc.gpsimd.wait_ge(s_idx, 16)
        nc.gpsimd.indirect_dma_start(
            out=t_g[:],
            out_offset=None,
            in_=xt[:],
            in_offset=bass.IndirectOffsetOnAxis(ap=t_idx[:, :1], axis=0),
        ).then_inc(s_g, 16)
        # d store here on gpsimd: issuing it on SP after store j0 made its
        # completion the last event; on Pool it issues right after the
        # gather lands and finishes well before the j-stores.
        nc.gpsimd.wait_ge(s_g, 16)
        nc.gpsimd.dma_start(out=d[:], in_=t_g[:]).then_inc(s_fin, 16)

        nc.vector.wait_ge(s_ld0, 16)
        nc.vector.tensor_mul(
            out=t[:, :J0], in0=t[:, :J0], in1=t[:, J0:H0]
        ).then_inc(s_mul, 1)
        nc.vector.wait_ge(s_ld1, 16)
        nc.vector.tensor_mul(
            out=t[:, H0 : H0 + J1],
            in0=t[:, H0 : H0 + J1],
            in1=t[:, H0 + J1 :],
        ).then_inc(s_mul, 1)

        nc.sync.wait_ge(s_mul, 1)
        nc.sync.dma_start(out=dram3d[:, :, :J0], in_=t[:, :J0]).then_inc(
            s_fin, 16
        )
        nc.scalar.wait_ge(s_mul, 2)
        nc.scalar.dma_start(
            out=dram3d[:, :, J0:], in_=t[:, H0 : H0 + J1]
        ).then_inc(s_fin, 16)
        nc.sync.wait_ge(s_fin, 48)

    return nc


_PROGRAM_CACHE: dict = {}


def _derive_structure(measure, fullmeasure):
    """Extract the 25 selected columns; verify the structural assumptions
    the kernel relies on. Returns None if they don't hold."""
    measure = np.asarray(measure, dtype=np.float32)
    if measure.shape != (OUT_DIM, IN_DIM):
        return None
    idx = measure.argmax(axis=1)
    onehot = np.zeros_like(measure)
    onehot[np.arange(OUT_DIM), idx] = 1.0
    if not np.array_equal(measure, onehot):
        return None
    diag = np.asarray(np.diagonal(fullmeasure), dtype=np.float32)
    expect = np.zeros(IN_DIM, dtype=np.float32)
    expect[idx] = 1.0
    if not np.array_equal(diag, expect):
        return None
    return tuple(int(f) for f in idx)


def kernel(input, measure, fullmeasure):
    from concourse.bass_utils import run_bass_kernel_spmd

    x = np.ascontiguousarray(np.asarray(input, dtype=np.float32))
    assert x.shape == (BS, IN_DIM), x.shape

    idx = _derive_structure(measure, fullmeasure)
    if idx is None:
        # Structure violated (cannot happen with the Metric construction):
        # fall back to the dense definition so the result is still correct.
        m = np.asarray(measure, dtype=np.float32)
        fm = np.asarray(fullmeasure, dtype=np.float32)
        return (x @ m.T, x @ fm.T)

    if "nc" not in _PROGRAM_CACHE:
        _PROGRAM_CACHE["nc"] = _build_program()
    nc = _PROGRAM_CACHE["nc"]

    mask = np.zeros(IN_DIM, dtype=np.float32)
    mask[list(idx)] = 1.0
    # mask in the (chunk*BPC + b, j) SBUF layout; identical for every b
    mz = np.ascontiguousarray(
        np.broadcast_to(
            mask.reshape(NCHUNK, 1, CHUNK), (NCHUNK, BPC, CHUNK)
        ).reshape(NPART, CHUNK)
    )
    idx_arr = np.asarray(idx, dtype=np.int32).reshape(OUT_DIM, 1)

    J0 = CHUNK // 2
    in_maps = []
    for k in range(N_CORES):
        shard = x[k * BPC : (k + 1) * BPC, :]  # (8, 9801)
        xz = shard.reshape(BPC, NCHUNK, CHUNK).transpose(1, 0, 2).reshape(
            NPART, CHUNK
        )
        # pack per row: [xz_j0 | mz_j0 | xz_j1 | mz_j1]
        xzmz = np.ascontiguousarray(
            np.concatenate(
                [xz[:, :J0], mz[:, :J0], xz[:, J0:], mz[:, J0:]], axis=1
            )
        )
        xt = np.ascontiguousarray(shard.T)
        in_maps.append({"xzmz": xzmz, "xt": xt, "idx": idx_arr})

    res = run_bass_kernel_spmd(nc, in_maps, core_ids=list(range(N_CORES)))
    d = np.concatenate([res.results[k]["d"].T for k in range(N_CORES)], axis=0)
    d_full = np.concatenate(
        [res.results[k]["dfull"] for k in range(N_CORES)], axis=0
    )
    return (d, d_full)
